# revision 53
# baseline (speedup 1.0000x reference)
"""Trainium2 Bass kernel for DifferentiableToposAttention.

Math:
  Q = sigmoid(x @ Wq.T + bq); K = sigmoid(x @ Wk.T + bk); V = x @ Wv.T + bv
  truth[q,k] = 1 - (1/D) sum_d relu(Q[q,d]-K[k,d]);  logit = 10*truth
  masked (k>q) positions get logit 0 exactly (softmax weight exp(0)=1).

Algorithmic core: piecewise-linear feature factorization.  With knots
t_p = p/T (p=0..T, h=1/T) and hat functions phi_p (interpolation in the
K variable is exact between knots; only the cell containing the kink of
relu carries O(h^2) error):

  relu(a-b) ~= sum_p phi_p(a) * relu(t_p - b)
  phi_p(a)  = -T * vt_p(a),  vt_p(a) = min(|a - t_p|, h) - h
  relu(t_p-b) = -(min(b, t_p) - t_p) = -m_p(b)

  sum_d relu(Q-K) ~= T * sum_{d,p} vt_p(Q[q,d]) * m_p(K[k,d]) =: T * SC

so the whole pairwise nonlinearity becomes one dense matmul with
contraction dim D*(T+1), run at 128x128 MACs/cycle on the PE instead of
the 128/cycle of a partition reduce.  logit = 10 - (10T/D)*SC.

Masking uses Z = (SC + D/T) * M1 (M1 host-supplied 0/1, SC <= 0), so
E = exp(+10T/D * Z) gives exp(logit) unmasked and exp(0)=1 masked, with
no bias corrections.  The last two knots' moving features are computed
on the scalar engine as relu(t_p - K) with a sign-flipped stationary,
balancing the DVE feature ladder.

Sharding: 8 cores; core c = (b, l) = (c//4, c%4) handles batch b, query
tiles l (keys 0..511 computed) and 4+l (keys 0..1023).  Shapes are
identical across cores (SPMD); causality is entirely in the M1 mask
data.  Keys >= 512 for tile A are all masked: weight-1 contributions
come from an all-ones stationary over V blocks 4..7 plus a +512
denominator constant.

Pipelining: score PSUM is built in 512-wide chunks in order A, B0, B1;
each chunk's Z -> exp -> EtT transposes -> AV matmuls overlap the next
chunk's score matmuls.
"""

import sys

for _p in ("/opt/trn_rl_repo",):
    if _p not in sys.path:
        sys.path.insert(0, _p)

import numpy as np

import concourse.bass as bass
import concourse.mybir as mybir
import concourse.tile as tile
from concourse import bacc
from concourse.bass import ts
from concourse.masks import make_identity
from concourse.bass_utils import run_bass_kernel_spmd

F32 = mybir.dt.float32
FP16 = mybir.dt.float16
AF = mybir.ActivationFunctionType
ALU = mybir.AluOpType

B, S, D = 2, 1024, 128
NCORES = 8
T = 6                    # knot count (h = 1/T); P = T+1 features per d
P = T + 1
H = 1.0 / T
POS_DT = float(D) / T    # Z = (SC + POS_DT) * M1  (SC <= 0)
EXP_SCALE = 10.0 * T / D
XKNOTS = (P - 2, P - 1)  # moving feature on ACT (relu) for these knots


def _build_program(masked: bool) -> bass.Bass:
    WA = 512 if masked else 1024   # computed key width, query tile A (tile l)
    WB = 1024                      # query tile B (tile 4+l)
    nc = bacc.Bacc()

    xbt_d = nc.declare_dram_parameter("xbt", [D, S], FP16, isOutput=False)
    xqt_d = nc.declare_dram_parameter("xqt", [D, 256], FP16, isOutput=False)
    wk_d = nc.declare_dram_parameter("wk", [D, D], FP16, isOutput=False)
    wq_d = nc.declare_dram_parameter("wq", [D, D], FP16, isOutput=False)
    wv_d = nc.declare_dram_parameter("wv", [D, D], FP16, isOutput=False)
    # bcat = [bk | bq | -bq | bvb(128 cols)]
    bcat_d = nc.declare_dram_parameter("bcat", [D, 3 + D], F32, isOutput=False)
    m1a_d = nc.declare_dram_parameter("m1a", [D, WA], FP16, isOutput=False)
    m1b_d = nc.declare_dram_parameter("m1b", [D, WB], FP16, isOutput=False)
    out_d = nc.declare_dram_parameter("out", [256, D], F32, isOutput=True)

    with tile.TileContext(nc) as tc:
        with tc.tile_pool(name="singles", bufs=1) as singles:
            wk_sb = singles.tile([128, 128], FP16)
            wq_sb = singles.tile([128, 128], FP16)
            wv_sb = singles.tile([128, 128], FP16)
            bcat_sb = singles.tile([128, 3 + 128], F32)
            xbt_sb = singles.tile([128, S], FP16)
            xqt_sb = singles.tile([128, 256], FP16)
            m1a_sb = singles.tile([128, WA], FP16)
            m1b_sb = singles.tile([128, WB], FP16)

            identity = singles.tile([128, 128], FP16)
            make_identity(nc, identity[:])
            warm512 = singles.tile([128, 512], FP16)
            nc.vector.memset(warm512[:], 1.0)
            ones128 = singles.tile([128, 128], FP16)
            nc.vector.memset(ones128[:], 1.0)

            # input DMAs spread across engine queues so they complete in
            # parallel; Q-path inputs (first consumers) lead their queues
            nc.sync.dma_start(out=xqt_sb[:], in_=xqt_d[:, :])
            nc.sync.dma_start(out=xbt_sb[:], in_=xbt_d[:, :])
            nc.scalar.dma_start(out=wq_sb[:], in_=wq_d[:, :])
            nc.scalar.dma_start(out=wk_sb[:], in_=wk_d[:, :])
            nc.gpsimd.dma_start(out=bcat_sb[:], in_=bcat_d[:, :])
            nc.gpsimd.dma_start(out=wv_sb[:], in_=wv_d[:, :])
            nc.sync.dma_start(out=m1a_sb[:], in_=m1a_d[:, :])
            nc.sync.dma_start(out=m1b_sb[:], in_=m1b_d[:, :])

            bk_sb = bcat_sb[:, 0:1]
            bq_sb = bcat_sb[:, 1:2]
            bqn_sb = bcat_sb[:, 2:3]
            bvb_sb = bcat_sb[:, 3:131]

            warm = singles.tile([128, 1], F32)
            # pull the sigmoid ACT table load to t~0 (no data deps)
            nc.scalar.activation(warm[:], ones128[:, 0:1], AF.Sigmoid)
            # per-partition bias constants t_p for the ACT relu knots
            tkn = singles.tile([128, len(XKNOTS)], F32)
            for i, p in enumerate(XKNOTS):
                nc.vector.memset(tkn[:, i:i + 1], p * H)

            KTb = singles.tile([128, S], FP16)     # sigmoid K^T  [d, k]
            QTb = singles.tile([128, 256], FP16)   # sigmoid Q^T  [d, q]
            QCb = singles.tile([128, 256], FP16)   # 1 - Q  (= sigmoid(-z))
            Vn = singles.tile([128, 8, 128], FP16)  # V (no bv)  [k, blk, e]
            mp = singles.tile([128, P, S], FP16)    # moving feats min(K,t)-t
            vt = singles.tile([128, P, 256], FP16)  # stationary feats

            # ---- phase 1: warm-up + K/Q projections ----
            with (
                tc.tile_pool(name="pwu", bufs=1, space="PSUM") as pwu,
                tc.tile_pool(name="pproj", bufs=1, space="PSUM") as pproj,
            ):
                # dummy matmuls bridge the input-DMA wait so the PE HAM
                # clock-gate is released (2.4 GHz) before real work
                wups = pwu.tile([128, 512], F32, tag="wup")
                for i in range(16):
                    nc.tensor.matmul(
                        wups[:, 0:256], identity[:], warm512[:, 0:256],
                        start=True, stop=True)

                psq = pproj.tile([128, 256], F32, tag="projq")
                nc.tensor.matmul(psq[:], wq_sb, xqt_sb[:])
                nc.scalar.activation(
                    QTb[:], psq[:], AF.Sigmoid, bias=bq_sb, scale=1.0)
                psk = pproj.tile([128, S], F32, tag="proj")
                for hh in range(2):
                    nc.tensor.matmul(
                        psk[:, ts(hh, 512)], wk_sb, xbt_sb[:, ts(hh, 512)])
                    nc.scalar.activation(
                        KTb[:, ts(hh, 512)], psk[:, ts(hh, 512)],
                        AF.Sigmoid, bias=bk_sb, scale=1.0)

            # ---- phase 2+3+4: features, chunked score -> exp -> AV ----
            with (
                tc.tile_pool(name="utmp", bufs=4) as utp,
                tc.tile_pool(name="psc", bufs=1, space="PSUM") as psc,
                tc.tile_pool(name="ezt", bufs=1) as ezt,
                tc.tile_pool(name="sml", bufs=1) as sml,
                tc.tile_pool(name="wts", bufs=4) as wtsp,
                tc.tile_pool(name="ob", bufs=2) as ob,
            ):
                QN = utp.tile([128, 256], FP16, tag="qn")
                nc.vector.tensor_scalar(QN[:], QTb[:], -1.0, None, ALU.mult)
                for p in range(P):
                    t_p = p * H
                    if p in XKNOTS:
                        # moving on ACT: g_p = relu(t_p - K) >= 0
                        tp_ap = tkn[:, XKNOTS.index(p):XKNOTS.index(p) + 1]
                        for hh in range(2):
                            nc.scalar.activation(
                                mp[:, p, ts(hh, 512)], KTb[:, ts(hh, 512)],
                                AF.Relu, bias=tp_ap, scale=-1.0)
                        # stationary vtNEG_p = max(min(Q-t_p-h,0),
                        #                          min(-Q+t_p-h,0)) <= 0
                        a = utp.tile([128, 256], FP16, tag="ua")
                        b = utp.tile([128, 256], FP16, tag="ub")
                        nc.vector.tensor_scalar(
                            a[:], QTb[:], t_p + H, 0.0, ALU.subtract, ALU.min)
                        nc.vector.tensor_scalar(
                            b[:], QN[:], t_p - H, 0.0, ALU.add, ALU.min)
                        nc.vector.tensor_max(vt[:, p, :], a[:], b[:])
                    else:
                        # moving on DVE: m_p = min(K, t_p) - t_p <= 0
                        for hh in range(2):
                            nc.vector.tensor_scalar(
                                mp[:, p, ts(hh, 512)], KTb[:, ts(hh, 512)],
                                t_p, t_p, ALU.min, ALU.subtract)
                        # stationary vtPOS_p = min(relu(t_p+h-Q),
                        #                          relu(Q-t_p+h)) >= 0
                        a = utp.tile([128, 256], FP16, tag="ua")
                        b = utp.tile([128, 256], FP16, tag="ub")
                        nc.vector.tensor_scalar(
                            a[:], QN[:], t_p + H, 0.0, ALU.add, ALU.max)
                        nc.vector.tensor_scalar(
                            b[:], QTb[:], t_p - H, 0.0, ALU.subtract, ALU.max)
                        nc.vector.tensor_tensor(
                            vt[:, p, :], a[:], b[:], ALU.min)

                # V projection + copies; exp table preload
                with tc.tile_pool(name="pvv", bufs=1, space="PSUM") as pvv:
                    for half in range(2):
                        psv = pvv.tile([128, 4, 128], F32, tag="vv")
                        for j4 in range(4):
                            j = half * 4 + j4
                            nc.tensor.matmul(
                                psv[:, j4, :], xbt_sb[:, ts(j, 128)], wv_sb)
                        nc.scalar.copy(Vn[:, ts(half, 4), :], psv[:])
                nc.scalar.activation(warm[:], QTb[:, 0:1], AF.Exp)

                pw_cm = tc.tile_pool(name="pw", bufs=3 if masked else 2,
                                     space="PSUM")
                po_cm = tc.tile_pool(name="po", bufs=1, space="PSUM")
                pw = pw_cm.__enter__()
                po = po_cm.__enter__()
                scA = psc.tile([128, WA], F32, tag="scA")
                scB = psc.tile([128, WB], F32, tag="scB")
                EA = ezt.tile([128, WA], FP16)
                EB = ezt.tile([128, WB], FP16)
                NCA, NCB = WA // 512, WB // 512
                rsA0 = sml.tile([128, 1], F32)
                rsA1 = sml.tile([128, 1], F32)
                rsB0 = sml.tile([128, 1], F32)
                rsB1 = sml.tile([128, 1], F32)
                rs2 = sml.tile([128, 1], F32)
                rs = {("A", 0): rsA0, ("A", 1): rsA1,
                      ("B", 0): rsB0, ("B", 1): rsB1}
                denA = sml.tile([128, 1], F32)
                denB = sml.tile([128, 1], F32)
                rcpA = sml.tile([128, 1], F32)
                rcpB = sml.tile([128, 1], F32)
                oA = po.tile([128, 128], F32, tag="oA")
                oB = po.tile([128, 128], F32, tag="oB")

                chunks = [("A", ca) for ca in range(NCA)]
                chunks += [("B", cb) for cb in range(NCB)]

                def emit_score(tile_id, ci):
                    sc = scA if tile_id == "A" else scB
                    qlo = 0 if tile_id == "A" else 128
                    for p in range(P):
                        nc.tensor.matmul(
                            sc[:, ts(ci, 512)],
                            vt[:, p, qlo:qlo + 128],
                            mp[:, p, ts(ci, 512)],
                            start=(p == 0), stop=(p == P - 1))

                def emit_zexp(tile_id, ci, nhalf):
                    sc, E = (scA, EA) if tile_id == "A" else (scB, EB)
                    m1 = m1a_sb if tile_id == "A" else m1b_sb
                    Z = utp.tile([128, 512], FP16, tag="z")
                    nc.vector.scalar_tensor_tensor(
                        out=Z[:], in0=sc[:, ts(ci, 512)], scalar=POS_DT,
                        in1=m1[:, ts(ci, 512)], op0=ALU.add, op1=ALU.mult)
                    rsc = rs[(tile_id, ci)]
                    for eh in range(nhalf):
                        w2 = 512 // nhalf
                        racc = rsc if eh == 0 else rs2
                        nc.scalar.activation(
                            E[:, ci * 512 + eh * w2:ci * 512 + (eh + 1) * w2],
                            Z[:, eh * w2:(eh + 1) * w2], AF.Exp,
                            scale=EXP_SCALE, accum_out=racc[:])
                    if nhalf == 2:
                        nc.vector.tensor_add(rsc[:], rsc[:], rs2[:])

                def emit_trav(tile_id, ci, close):
                    E = EA if tile_id == "A" else EB
                    o = oA if tile_id == "A" else oB
                    for j4 in range(4):
                        j = ci * 4 + j4
                        pwt = pw.tile([128, 128], FP16, tag="wt")
                        nc.tensor.transpose(
                            pwt[:], E[:, ts(j, 128)], identity[:])
                        wtile = wtsp.tile([128, 128], FP16, tag="wts")
                        if j % 2 == 0:
                            nc.scalar.copy(wtile[:], pwt[:])
                        else:
                            nc.vector.tensor_copy(wtile[:], pwt[:])
                        nc.tensor.matmul(
                            o[:], wtile[:], Vn[:, j, :],
                            start=(j == 0), stop=(close and j4 == 3))

                def emit_out(tile_id, o, rcp, r0, split=False):
                    ores = ob.tile([128, 128], F32, tag="ores")
                    if split:
                        # halves so the first store overlaps the second stt
                        nc.vector.scalar_tensor_tensor(
                            out=ores[:, 0:64], in0=o[:, 0:64], scalar=rcp[:],
                            in1=bvb_sb[:, 0:64], op0=ALU.mult, op1=ALU.add)
                        nc.sync.dma_start(
                            out=out_d[r0:r0 + 128, 0:64], in_=ores[:, 0:64])
                        nc.vector.scalar_tensor_tensor(
                            out=ores[:, 64:128], in0=o[:, 64:128],
                            scalar=rcp[:], in1=bvb_sb[:, 64:128],
                            op0=ALU.mult, op1=ALU.add)
                        nc.scalar.dma_start(
                            out=out_d[r0:r0 + 128, 64:128],
                            in_=ores[:, 64:128])
                    else:
                        nc.vector.scalar_tensor_tensor(
                            out=ores[:], in0=o[:], scalar=rcp[:],
                            in1=bvb_sb, op0=ALU.mult, op1=ALU.add)
                        nc.sync.dma_start(
                            out=out_d[r0:r0 + 128, :], in_=ores[:])

                def emit_a_epilogue():
                    if masked:
                        # masked tail keys 512..1023: weight-1
                        for j in range(4, 8):
                            nc.tensor.matmul(
                                oA[:], ones128[:], Vn[:, j, :],
                                start=False, stop=(j == 7))
                        nc.vector.tensor_scalar(
                            denA[:], rs[("A", 0)][:], float(S - WA),
                            None, ALU.add)
                    else:
                        nc.vector.tensor_add(
                            denA[:], rs[("A", 0)][:], rs[("A", 1)][:])
                    nc.vector.reciprocal(rcpA[:], denA[:])
                    emit_out("A", oA, rcpA, 0)

                # chunk-inline tails (scheduler interleaves them with the
                # next chunk's scores); final chunk's transposes go after
                # the A epilogue so its Z/exp keep queue priority
                for (tile_id, ci) in chunks[:-1]:
                    last = ci == (NCA if tile_id == "A" else NCB) - 1
                    emit_score(tile_id, ci)
                    emit_zexp(tile_id, ci, 1)
                    emit_trav(tile_id, ci, close=(last and not
                                                  (tile_id == "A" and masked)))
                emit_score("B", NCB - 1)
                emit_zexp("B", NCB - 1, 2)
                emit_a_epilogue()
                emit_trav("B", NCB - 1, close=True)
                nc.vector.tensor_add(
                    denB[:], rs[("B", 0)][:], rs[("B", 1)][:])
                nc.vector.reciprocal(rcpB[:], denB[:])
                emit_out("B", oB, rcpB, 128, split=True)
                po_cm.__exit__(None, None, None)
                pw_cm.__exit__(None, None, None)

    nc.finalize()
    return nc


_PROG_CACHE: dict[bool, bass.Bass] = {}


def _get_program(masked: bool) -> bass.Bass:
    if masked not in _PROG_CACHE:
        _PROG_CACHE[masked] = _build_program(masked)
    return _PROG_CACHE[masked]


def build_in_maps(x, Wq, bq, Wk, bk, Wv, bv, masked):
    wkt = np.ascontiguousarray(Wk.T.astype(np.float16))
    wqt = np.ascontiguousarray(Wq.T.astype(np.float16))
    wvt = np.ascontiguousarray(Wv.T.astype(np.float16))
    bcat = np.ascontiguousarray(
        np.concatenate(
            [bk.reshape(D, 1), bq.reshape(D, 1), -bq.reshape(D, 1),
             np.tile(bv.reshape(1, D), (D, 1))], axis=1).astype(np.float32))
    WA = 512 if masked else 1024
    WB = 1024
    kidx = np.arange(S)
    in_maps = []
    for c in range(NCORES):
        b, l = divmod(c, 4)
        xb16 = x[b].astype(np.float16)
        xbt = np.ascontiguousarray(xb16.T)
        rows = np.concatenate(
            [128 * l + np.arange(128), 128 * (4 + l) + np.arange(128)])
        xqt = np.ascontiguousarray(xb16[rows].T)
        if masked:
            qa = (128 * l + np.arange(128))[:, None]
            qb = (128 * (4 + l) + np.arange(128))[:, None]
            m1a = (kidx[None, :WA] <= qa).astype(np.float16)
            m1b = (kidx[None, :WB] <= qb).astype(np.float16)
        else:
            m1a = np.ones((128, WA), np.float16)
            m1b = np.ones((128, WB), np.float16)
        in_maps.append({
            "xbt": xbt, "xqt": xqt, "wk": wkt, "wq": wqt, "wv": wvt,
            "bcat": bcat,
            "m1a": np.ascontiguousarray(m1a),
            "m1b": np.ascontiguousarray(m1b),
        })
    return in_maps


def assemble_out(results, masked):
    out = np.empty((B, S, D), dtype=np.float32)
    for c in range(NCORES):
        b, l = divmod(c, 4)
        res = results[c]["out"]
        out[b, 128 * l:128 * (l + 1)] = res[0:128]
        out[b, 128 * (4 + l):128 * (5 + l)] = res[128:256]
    return out


def kernel(x, Wq, bq, Wk, bk, Wv, bv, apply_causal_mask):
    x = np.ascontiguousarray(np.asarray(x, dtype=np.float32))
    Wq = np.asarray(Wq, dtype=np.float32)
    Wk = np.asarray(Wk, dtype=np.float32)
    Wv = np.asarray(Wv, dtype=np.float32)
    bq = np.asarray(bq, dtype=np.float32)
    bk = np.asarray(bk, dtype=np.float32)
    bv = np.asarray(bv, dtype=np.float32)
    masked = bool(int(np.asarray(apply_causal_mask)))

    nc = _get_program(masked)
    in_maps = build_in_maps(x, Wq, bq, Wk, bk, Wv, bv, masked)
    res = run_bass_kernel_spmd(nc, in_maps, list(range(NCORES))).results
    return assemble_out(res, masked)


# revision 54
# speedup vs baseline: 1.0007x; 1.0007x over previous
"""Trainium2 Bass kernel for DifferentiableToposAttention.

Math:
  Q = sigmoid(x @ Wq.T + bq); K = sigmoid(x @ Wk.T + bk); V = x @ Wv.T + bv
  truth[q,k] = 1 - (1/D) sum_d relu(Q[q,d]-K[k,d]);  logit = 10*truth
  masked (k>q) positions get logit 0 exactly (softmax weight exp(0)=1).

Algorithmic core: piecewise-linear feature factorization.  With knots
t_p = p/T (p=0..T, h=1/T) and hat functions phi_p (interpolation in the
K variable is exact between knots; only the cell containing the kink of
relu carries O(h^2) error):

  relu(a-b) ~= sum_p phi_p(a) * relu(t_p - b)
  phi_p(a)  = -T * vt_p(a),  vt_p(a) = min(|a - t_p|, h) - h
  relu(t_p-b) = -(min(b, t_p) - t_p) = -m_p(b)

  sum_d relu(Q-K) ~= T * sum_{d,p} vt_p(Q[q,d]) * m_p(K[k,d]) =: T * SC

so the whole pairwise nonlinearity becomes one dense matmul with
contraction dim D*(T+1), run at 128x128 MACs/cycle on the PE instead of
the 128/cycle of a partition reduce.  logit = 10 - (10T/D)*SC.

Masking uses Z = (SC + D/T) * M1 (M1 host-supplied 0/1, SC <= 0), so
E = exp(+10T/D * Z) gives exp(logit) unmasked and exp(0)=1 masked, with
no bias corrections.  The last two knots' moving features are computed
on the scalar engine as relu(t_p - K) with a sign-flipped stationary,
balancing the DVE feature ladder.

Sharding: 8 cores; core c = (b, l) = (c//4, c%4) handles batch b, query
tiles l (keys 0..511 computed) and 4+l (keys 0..1023).  Shapes are
identical across cores (SPMD); causality is entirely in the M1 mask
data.  Keys >= 512 for tile A are all masked: weight-1 contributions
come from an all-ones stationary over V blocks 4..7 plus a +512
denominator constant.

Pipelining: score PSUM is built in 512-wide chunks in order A, B0, B1;
each chunk's Z -> exp -> EtT transposes -> AV matmuls overlap the next
chunk's score matmuls.
"""

import sys

for _p in ("/opt/trn_rl_repo",):
    if _p not in sys.path:
        sys.path.insert(0, _p)

import numpy as np

import concourse.bass as bass
import concourse.mybir as mybir
import concourse.tile as tile
from concourse import bacc
from concourse.bass import ts
from concourse.masks import make_identity
from concourse.bass_utils import run_bass_kernel_spmd

F32 = mybir.dt.float32
FP16 = mybir.dt.float16
AF = mybir.ActivationFunctionType
ALU = mybir.AluOpType

B, S, D = 2, 1024, 128
NCORES = 8
T = 6                    # knot count (h = 1/T); P = T+1 features per d
P = T + 1
H = 1.0 / T
POS_DT = float(D) / T    # Z = (SC + POS_DT) * M1  (SC <= 0)
EXP_SCALE = 10.0 * T / D
XKNOTS = (P - 2, P - 1)  # moving feature on ACT (relu) for these knots


def _build_program(masked: bool) -> bass.Bass:
    WA = 512 if masked else 1024   # computed key width, query tile A (tile l)
    WB = 1024                      # query tile B (tile 4+l)
    nc = bacc.Bacc()

    xbt_d = nc.declare_dram_parameter("xbt", [D, S], FP16, isOutput=False)
    xqt_d = nc.declare_dram_parameter("xqt", [D, 256], FP16, isOutput=False)
    wk_d = nc.declare_dram_parameter("wk", [D, D], FP16, isOutput=False)
    wq_d = nc.declare_dram_parameter("wq", [D, D], FP16, isOutput=False)
    wv_d = nc.declare_dram_parameter("wv", [D, D], FP16, isOutput=False)
    # bcat = [bk | bq | -bq | bvb(128 cols)]
    bcat_d = nc.declare_dram_parameter("bcat", [D, 3 + D], F32, isOutput=False)
    m1a_d = nc.declare_dram_parameter("m1a", [D, WA], FP16, isOutput=False)
    m1b_d = nc.declare_dram_parameter("m1b", [D, WB], FP16, isOutput=False)
    out_d = nc.declare_dram_parameter("out", [256, D], F32, isOutput=True)

    with tile.TileContext(nc) as tc:
        with tc.tile_pool(name="singles", bufs=1) as singles:
            wk_sb = singles.tile([128, 128], FP16)
            wq_sb = singles.tile([128, 128], FP16)
            wv_sb = singles.tile([128, 128], FP16)
            bcat_sb = singles.tile([128, 3 + 128], F32)
            xbt_sb = singles.tile([128, S], FP16)
            xqt_sb = singles.tile([128, 256], FP16)
            m1a_sb = singles.tile([128, WA], FP16)
            m1b_sb = singles.tile([128, WB], FP16)

            identity = singles.tile([128, 128], FP16)
            make_identity(nc, identity[:])
            warm512 = singles.tile([128, 512], FP16)
            nc.vector.memset(warm512[:], 1.0)
            ones128 = singles.tile([128, 128], FP16)
            nc.vector.memset(ones128[:], 1.0)

            # input DMAs spread across engine queues so they complete in
            # parallel; Q-path inputs (first consumers) lead their queues
            nc.sync.dma_start(out=xqt_sb[:], in_=xqt_d[:, :])
            nc.sync.dma_start(out=xbt_sb[:], in_=xbt_d[:, :])
            nc.scalar.dma_start(out=wq_sb[:], in_=wq_d[:, :])
            nc.scalar.dma_start(out=wk_sb[:], in_=wk_d[:, :])
            nc.gpsimd.dma_start(out=bcat_sb[:], in_=bcat_d[:, :])
            nc.gpsimd.dma_start(out=wv_sb[:], in_=wv_d[:, :])
            nc.sync.dma_start(out=m1a_sb[:], in_=m1a_d[:, :])
            nc.sync.dma_start(out=m1b_sb[:], in_=m1b_d[:, :])

            bk_sb = bcat_sb[:, 0:1]
            bq_sb = bcat_sb[:, 1:2]
            bqn_sb = bcat_sb[:, 2:3]
            bvb_sb = bcat_sb[:, 3:131]

            warm = singles.tile([128, 1], F32)
            # pull the sigmoid ACT table load to t~0 (no data deps)
            nc.scalar.activation(warm[:], ones128[:, 0:1], AF.Sigmoid)
            # per-partition bias constants t_p for the ACT relu knots
            tkn = singles.tile([128, len(XKNOTS)], F32)
            for i, p in enumerate(XKNOTS):
                nc.vector.memset(tkn[:, i:i + 1], p * H)

            KTb = singles.tile([128, S], FP16)     # sigmoid K^T  [d, k]
            QTb = singles.tile([128, 256], FP16)   # sigmoid Q^T  [d, q]
            QCb = singles.tile([128, 256], FP16)   # 1 - Q  (= sigmoid(-z))
            Vn = singles.tile([128, 8, 128], FP16)  # V (no bv)  [k, blk, e]
            mp = singles.tile([128, P, S], FP16)    # moving feats min(K,t)-t
            vt = singles.tile([128, P, 256], FP16)  # stationary feats

            # ---- phase 1: warm-up + K/Q projections ----
            with (
                tc.tile_pool(name="pwu", bufs=1, space="PSUM") as pwu,
                tc.tile_pool(name="pproj", bufs=1, space="PSUM") as pproj,
            ):
                # dummy matmuls bridge the input-DMA wait so the PE HAM
                # clock-gate is released (2.4 GHz) before real work
                wups = pwu.tile([128, 512], F32, tag="wup")
                for i in range(16):
                    nc.tensor.matmul(
                        wups[:, 0:256], identity[:], warm512[:, 0:256],
                        start=True, stop=True)

                psq = pproj.tile([128, 256], F32, tag="projq")
                nc.tensor.matmul(psq[:], wq_sb, xqt_sb[:])
                nc.scalar.activation(
                    QTb[:], psq[:], AF.Sigmoid, bias=bq_sb, scale=1.0)
                psk = pproj.tile([128, S], F32, tag="proj")
                for hh in range(2):
                    nc.tensor.matmul(
                        psk[:, ts(hh, 512)], wk_sb, xbt_sb[:, ts(hh, 512)])
                    nc.scalar.activation(
                        KTb[:, ts(hh, 512)], psk[:, ts(hh, 512)],
                        AF.Sigmoid, bias=bk_sb, scale=1.0)

            # ---- phase 2+3+4: features, chunked score -> exp -> AV ----
            with (
                tc.tile_pool(name="utmp", bufs=4) as utp,
                tc.tile_pool(name="psc", bufs=1, space="PSUM") as psc,
                tc.tile_pool(name="ezt", bufs=1) as ezt,
                tc.tile_pool(name="sml", bufs=1) as sml,
                tc.tile_pool(name="wts", bufs=4) as wtsp,
                tc.tile_pool(name="ob", bufs=2) as ob,
            ):
                QN = utp.tile([128, 256], FP16, tag="qn")
                nc.vector.tensor_scalar(QN[:], QTb[:], -1.0, None, ALU.mult)
                for p in range(P):
                    t_p = p * H
                    if p in XKNOTS:
                        # moving on ACT: g_p = relu(t_p - K) >= 0
                        tp_ap = tkn[:, XKNOTS.index(p):XKNOTS.index(p) + 1]
                        for hh in range(2):
                            nc.scalar.activation(
                                mp[:, p, ts(hh, 512)], KTb[:, ts(hh, 512)],
                                AF.Relu, bias=tp_ap, scale=-1.0)
                        # stationary vtNEG_p = max(min(Q-t_p-h,0),
                        #                          min(-Q+t_p-h,0)) <= 0
                        a = utp.tile([128, 256], FP16, tag="ua")
                        b = utp.tile([128, 256], FP16, tag="ub")
                        nc.vector.tensor_scalar(
                            a[:], QTb[:], t_p + H, 0.0, ALU.subtract, ALU.min)
                        nc.vector.tensor_scalar(
                            b[:], QN[:], t_p - H, 0.0, ALU.add, ALU.min)
                        nc.vector.tensor_max(vt[:, p, :], a[:], b[:])
                    else:
                        # moving on DVE: m_p = min(K, t_p) - t_p <= 0
                        for hh in range(2):
                            nc.vector.tensor_scalar(
                                mp[:, p, ts(hh, 512)], KTb[:, ts(hh, 512)],
                                t_p, t_p, ALU.min, ALU.subtract)
                        # stationary vtPOS_p = min(relu(t_p+h-Q),
                        #                          relu(Q-t_p+h)) >= 0
                        a = utp.tile([128, 256], FP16, tag="ua")
                        b = utp.tile([128, 256], FP16, tag="ub")
                        nc.vector.tensor_scalar(
                            a[:], QN[:], t_p + H, 0.0, ALU.add, ALU.max)
                        nc.vector.tensor_scalar(
                            b[:], QTb[:], t_p - H, 0.0, ALU.subtract, ALU.max)
                        nc.vector.tensor_tensor(
                            vt[:, p, :], a[:], b[:], ALU.min)

                # V projection + copies; exp table preload
                with tc.tile_pool(name="pvv", bufs=1, space="PSUM") as pvv:
                    for half in range(2):
                        psv = pvv.tile([128, 4, 128], F32, tag="vv")
                        for j4 in range(4):
                            j = half * 4 + j4
                            nc.tensor.matmul(
                                psv[:, j4, :], xbt_sb[:, ts(j, 128)], wv_sb)
                        nc.scalar.copy(Vn[:, ts(half, 4), :], psv[:])
                nc.scalar.activation(warm[:], QTb[:, 0:1], AF.Exp)

                pw_cm = tc.tile_pool(name="pw", bufs=3 if masked else 2,
                                     space="PSUM")
                po_cm = tc.tile_pool(name="po", bufs=1, space="PSUM")
                pw = pw_cm.__enter__()
                po = po_cm.__enter__()
                scA = psc.tile([128, WA], F32, tag="scA")
                scB = psc.tile([128, WB], F32, tag="scB")
                EA = ezt.tile([128, WA], FP16)
                EB = ezt.tile([128, WB], FP16)
                NCA, NCB = WA // 512, WB // 512
                rsA0 = sml.tile([128, 1], F32)
                rsA1 = sml.tile([128, 1], F32)
                rsB0 = sml.tile([128, 1], F32)
                rsB1 = sml.tile([128, 1], F32)
                rs2 = sml.tile([128, 1], F32)
                rs = {("A", 0): rsA0, ("A", 1): rsA1,
                      ("B", 0): rsB0, ("B", 1): rsB1}
                denA = sml.tile([128, 1], F32)
                denB = sml.tile([128, 1], F32)
                rcpA = sml.tile([128, 1], F32)
                rcpB = sml.tile([128, 1], F32)
                oA = po.tile([128, 128], F32, tag="oA")
                oB = po.tile([128, 128], F32, tag="oB")

                chunks = [("A", ca) for ca in range(NCA)]
                chunks += [("B", cb) for cb in range(NCB)]

                def emit_score(tile_id, ci):
                    sc = scA if tile_id == "A" else scB
                    qlo = 0 if tile_id == "A" else 128
                    for p in range(P):
                        nc.tensor.matmul(
                            sc[:, ts(ci, 512)],
                            vt[:, p, qlo:qlo + 128],
                            mp[:, p, ts(ci, 512)],
                            start=(p == 0), stop=(p == P - 1))

                def emit_zexp(tile_id, ci, nhalf):
                    sc, E = (scA, EA) if tile_id == "A" else (scB, EB)
                    m1 = m1a_sb if tile_id == "A" else m1b_sb
                    Z = utp.tile([128, 512], FP16, tag="z")
                    nc.vector.scalar_tensor_tensor(
                        out=Z[:], in0=sc[:, ts(ci, 512)], scalar=POS_DT,
                        in1=m1[:, ts(ci, 512)], op0=ALU.add, op1=ALU.mult)
                    rsc = rs[(tile_id, ci)]
                    for eh in range(nhalf):
                        w2 = 512 // nhalf
                        racc = rsc if eh == 0 else rs2
                        nc.scalar.activation(
                            E[:, ci * 512 + eh * w2:ci * 512 + (eh + 1) * w2],
                            Z[:, eh * w2:(eh + 1) * w2], AF.Exp,
                            scale=EXP_SCALE, accum_out=racc[:])
                    if nhalf == 2:
                        nc.vector.tensor_add(rsc[:], rsc[:], rs2[:])

                def emit_trav(tile_id, ci, close):
                    E = EA if tile_id == "A" else EB
                    o = oA if tile_id == "A" else oB
                    for j4 in range(4):
                        j = ci * 4 + j4
                        pwt = pw.tile([128, 128], FP16, tag="wt")
                        nc.tensor.transpose(
                            pwt[:], E[:, ts(j, 128)], identity[:])
                        wtile = wtsp.tile([128, 128], FP16, tag="wts")
                        if j % 2 == 0:
                            nc.scalar.copy(wtile[:], pwt[:])
                        else:
                            nc.vector.tensor_copy(wtile[:], pwt[:])
                        nc.tensor.matmul(
                            o[:], wtile[:], Vn[:, j, :],
                            start=(j == 0), stop=(close and j4 == 3))

                def emit_out(tile_id, o, rcp, r0, split=False):
                    ores = ob.tile([128, 128], F32, tag="ores")
                    if split:
                        # halves so the first store overlaps the second stt
                        nc.vector.scalar_tensor_tensor(
                            out=ores[:, 0:64], in0=o[:, 0:64], scalar=rcp[:],
                            in1=bvb_sb[:, 0:64], op0=ALU.mult, op1=ALU.add)
                        nc.sync.dma_start(
                            out=out_d[r0:r0 + 128, 0:64], in_=ores[:, 0:64])
                        nc.vector.scalar_tensor_tensor(
                            out=ores[:, 64:128], in0=o[:, 64:128],
                            scalar=rcp[:], in1=bvb_sb[:, 64:128],
                            op0=ALU.mult, op1=ALU.add)
                        nc.scalar.dma_start(
                            out=out_d[r0:r0 + 128, 64:128],
                            in_=ores[:, 64:128])
                    else:
                        nc.vector.scalar_tensor_tensor(
                            out=ores[:], in0=o[:], scalar=rcp[:],
                            in1=bvb_sb, op0=ALU.mult, op1=ALU.add)
                        nc.sync.dma_start(
                            out=out_d[r0:r0 + 128, :], in_=ores[:])

                def emit_a_epilogue():
                    if masked:
                        # masked tail keys 512..1023: weight-1
                        for j in range(4, 8):
                            nc.tensor.matmul(
                                oA[:], ones128[:], Vn[:, j, :],
                                start=False, stop=(j == 7))
                        nc.vector.tensor_scalar(
                            denA[:], rs[("A", 0)][:], float(S - WA),
                            None, ALU.add)
                    else:
                        nc.vector.tensor_add(
                            denA[:], rs[("A", 0)][:], rs[("A", 1)][:])
                    nc.vector.reciprocal(rcpA[:], denA[:])
                    emit_out("A", oA, rcpA, 0)

                # knot-major scores: every chunk advances in lockstep
                # with the feature ladder, so all chunks complete when the
                # last knot's features land (instead of B0/B1 re-sweeping
                # all knots serially after chunk A); B0/B1 share their
                # stationary per knot
                for p in range(P):
                    for (tile_id, ci) in chunks:
                        sc = scA if tile_id == "A" else scB
                        qlo = 0 if tile_id == "A" else 128
                        nc.tensor.matmul(
                            sc[:, ts(ci, 512)],
                            vt[:, p, qlo:qlo + 128],
                            mp[:, p, ts(ci, 512)],
                            start=(p == 0), stop=(p == P - 1))
                for (tile_id, ci) in chunks[:-1]:
                    last = ci == (NCA if tile_id == "A" else NCB) - 1
                    emit_zexp(tile_id, ci, 1)
                    emit_trav(tile_id, ci, close=(last and not
                                                  (tile_id == "A" and masked)))
                emit_zexp("B", NCB - 1, 2)
                emit_a_epilogue()
                emit_trav("B", NCB - 1, close=True)
                nc.vector.tensor_add(
                    denB[:], rs[("B", 0)][:], rs[("B", 1)][:])
                nc.vector.reciprocal(rcpB[:], denB[:])
                emit_out("B", oB, rcpB, 128, split=True)
                po_cm.__exit__(None, None, None)
                pw_cm.__exit__(None, None, None)

    nc.finalize()
    return nc


_PROG_CACHE: dict[bool, bass.Bass] = {}


def _get_program(masked: bool) -> bass.Bass:
    if masked not in _PROG_CACHE:
        _PROG_CACHE[masked] = _build_program(masked)
    return _PROG_CACHE[masked]


def build_in_maps(x, Wq, bq, Wk, bk, Wv, bv, masked):
    wkt = np.ascontiguousarray(Wk.T.astype(np.float16))
    wqt = np.ascontiguousarray(Wq.T.astype(np.float16))
    wvt = np.ascontiguousarray(Wv.T.astype(np.float16))
    bcat = np.ascontiguousarray(
        np.concatenate(
            [bk.reshape(D, 1), bq.reshape(D, 1), -bq.reshape(D, 1),
             np.tile(bv.reshape(1, D), (D, 1))], axis=1).astype(np.float32))
    WA = 512 if masked else 1024
    WB = 1024
    kidx = np.arange(S)
    in_maps = []
    for c in range(NCORES):
        b, l = divmod(c, 4)
        xb16 = x[b].astype(np.float16)
        xbt = np.ascontiguousarray(xb16.T)
        rows = np.concatenate(
            [128 * l + np.arange(128), 128 * (4 + l) + np.arange(128)])
        xqt = np.ascontiguousarray(xb16[rows].T)
        if masked:
            qa = (128 * l + np.arange(128))[:, None]
            qb = (128 * (4 + l) + np.arange(128))[:, None]
            m1a = (kidx[None, :WA] <= qa).astype(np.float16)
            m1b = (kidx[None, :WB] <= qb).astype(np.float16)
        else:
            m1a = np.ones((128, WA), np.float16)
            m1b = np.ones((128, WB), np.float16)
        in_maps.append({
            "xbt": xbt, "xqt": xqt, "wk": wkt, "wq": wqt, "wv": wvt,
            "bcat": bcat,
            "m1a": np.ascontiguousarray(m1a),
            "m1b": np.ascontiguousarray(m1b),
        })
    return in_maps


def assemble_out(results, masked):
    out = np.empty((B, S, D), dtype=np.float32)
    for c in range(NCORES):
        b, l = divmod(c, 4)
        res = results[c]["out"]
        out[b, 128 * l:128 * (l + 1)] = res[0:128]
        out[b, 128 * (4 + l):128 * (5 + l)] = res[128:256]
    return out


def kernel(x, Wq, bq, Wk, bk, Wv, bv, apply_causal_mask):
    x = np.ascontiguousarray(np.asarray(x, dtype=np.float32))
    Wq = np.asarray(Wq, dtype=np.float32)
    Wk = np.asarray(Wk, dtype=np.float32)
    Wv = np.asarray(Wv, dtype=np.float32)
    bq = np.asarray(bq, dtype=np.float32)
    bk = np.asarray(bk, dtype=np.float32)
    bv = np.asarray(bv, dtype=np.float32)
    masked = bool(int(np.asarray(apply_causal_mask)))

    nc = _get_program(masked)
    in_maps = build_in_maps(x, Wq, bq, Wk, bk, Wv, bv, masked)
    res = run_bass_kernel_spmd(nc, in_maps, list(range(NCORES))).results
    return assemble_out(res, masked)


# revision 55
# speedup vs baseline: 1.1536x; 1.1529x over previous
"""Trainium2 Bass kernel for DifferentiableToposAttention.

Math:
  Q = sigmoid(x @ Wq.T + bq); K = sigmoid(x @ Wk.T + bk); V = x @ Wv.T + bv
  truth[q,k] = 1 - (1/D) sum_d relu(Q[q,d]-K[k,d]);  logit = 10*truth
  masked (k>q) positions get logit 0 exactly (softmax weight exp(0)=1).

Algorithmic core: piecewise-linear feature factorization.  With knots
t_p = p/T (p=0..T, h=1/T) and hat functions phi_p (interpolation in the
K variable is exact between knots; only the cell containing the kink of
relu carries O(h^2) error):

  relu(a-b) ~= sum_p phi_p(a) * relu(t_p - b)
  phi_p(a)  = -T * vt_p(a),  vt_p(a) = min(|a - t_p|, h) - h
  relu(t_p-b) = -(min(b, t_p) - t_p) = -m_p(b)

  sum_d relu(Q-K) ~= T * sum_{d,p} vt_p(Q[q,d]) * m_p(K[k,d]) =: T * SC

so the whole pairwise nonlinearity becomes one dense matmul with
contraction dim D*(T+1), run at 128x128 MACs/cycle on the PE instead of
the 128/cycle of a partition reduce.  logit = 10 - (10T/D)*SC.

Masking uses Z = (SC + D/T) * M1 (M1 host-supplied 0/1, SC <= 0), so
E = exp(+10T/D * Z) gives exp(logit) unmasked and exp(0)=1 masked, with
no bias corrections.  The last two knots' moving features are computed
on the scalar engine as relu(t_p - K) with a sign-flipped stationary,
balancing the DVE feature ladder.

Sharding: 8 cores; core c = (b, l) = (c//4, c%4) handles batch b, query
tiles l (keys 0..511 computed) and 4+l (keys 0..1023).  Shapes are
identical across cores (SPMD); causality is entirely in the M1 mask
data.  Keys >= 512 for tile A are all masked: weight-1 contributions
come from an all-ones stationary over V blocks 4..7 plus a +512
denominator constant.

Pipelining: score PSUM is built in 512-wide chunks in order A, B0, B1;
each chunk's Z -> exp -> EtT transposes -> AV matmuls overlap the next
chunk's score matmuls.
"""

import sys

for _p in ("/opt/trn_rl_repo",):
    if _p not in sys.path:
        sys.path.insert(0, _p)

import numpy as np

import concourse.bass as bass
import concourse.mybir as mybir
import concourse.tile as tile
from concourse import bacc
from concourse.bass import ts
from concourse.masks import make_identity
from concourse.bass_utils import run_bass_kernel_spmd

F32 = mybir.dt.float32
FP16 = mybir.dt.float16
AF = mybir.ActivationFunctionType
ALU = mybir.AluOpType

B, S, D = 2, 1024, 128
NCORES = 8
T = 6                    # knot count (h = 1/T); P = T+1 features per d
P = T + 1
H = 1.0 / T
POS_DT = float(D) / T    # Z = (SC + POS_DT) * M1  (SC <= 0)
EXP_SCALE = 10.0 * T / D
XKNOTS = (P - 2, P - 1)  # moving feature on ACT (relu) for these knots


def _build_program(masked: bool) -> bass.Bass:
    WA = 512 if masked else 1024   # computed key width, query tile A (tile l)
    WB = 1024                      # query tile B (tile 4+l)
    nc = bacc.Bacc()

    xbt_d = nc.declare_dram_parameter("xbt", [D, S], FP16, isOutput=False)
    xqt_d = nc.declare_dram_parameter("xqt", [D, 256], FP16, isOutput=False)
    wk_d = nc.declare_dram_parameter("wk", [D, D], FP16, isOutput=False)
    wq_d = nc.declare_dram_parameter("wq", [D, D], FP16, isOutput=False)
    wv_d = nc.declare_dram_parameter("wv", [D, D], FP16, isOutput=False)
    # bcat = [bk | bq | -bq | bvb(128 cols)]
    bcat_d = nc.declare_dram_parameter("bcat", [D, 3 + D], F32, isOutput=False)
    m1a_d = nc.declare_dram_parameter("m1a", [D, WA], FP16, isOutput=False)
    m1b_d = nc.declare_dram_parameter("m1b", [D, WB], FP16, isOutput=False)
    out_d = nc.declare_dram_parameter("out", [256, D], F32, isOutput=True)

    with tile.TileContext(nc) as tc:
        with tc.tile_pool(name="singles", bufs=1) as singles:
            wk_sb = singles.tile([128, 128], FP16)
            wq_sb = singles.tile([128, 128], FP16)
            wv_sb = singles.tile([128, 128], FP16)
            bcat_sb = singles.tile([128, 3 + 128], F32)
            xbt_sb = singles.tile([128, S], FP16)
            xqt_sb = singles.tile([128, 256], FP16)
            m1a_sb = singles.tile([128, WA], FP16)
            m1b_sb = singles.tile([128, WB], FP16)

            identity = singles.tile([128, 128], FP16)
            make_identity(nc, identity[:])
            warm512 = singles.tile([128, 512], FP16)
            nc.vector.memset(warm512[:], 1.0)
            ones128 = singles.tile([128, 128], FP16)
            nc.vector.memset(ones128[:], 1.0)

            # input DMAs spread across engine queues so they complete in
            # parallel; Q-path inputs (first consumers) lead their queues
            nc.sync.dma_start(out=xqt_sb[:], in_=xqt_d[:, :])
            nc.sync.dma_start(out=xbt_sb[:], in_=xbt_d[:, :])
            nc.scalar.dma_start(out=wq_sb[:], in_=wq_d[:, :])
            nc.scalar.dma_start(out=wk_sb[:], in_=wk_d[:, :])
            nc.gpsimd.dma_start(out=bcat_sb[:], in_=bcat_d[:, :])
            nc.gpsimd.dma_start(out=wv_sb[:], in_=wv_d[:, :])
            nc.sync.dma_start(out=m1a_sb[:], in_=m1a_d[:, :])
            nc.sync.dma_start(out=m1b_sb[:], in_=m1b_d[:, :])

            bk_sb = bcat_sb[:, 0:1]
            bq_sb = bcat_sb[:, 1:2]
            bqn_sb = bcat_sb[:, 2:3]
            bvb_sb = bcat_sb[:, 3:131]

            warm = singles.tile([128, 1], F32)
            # pull the sigmoid ACT table load to t~0 (no data deps)
            nc.scalar.activation(warm[:], ones128[:, 0:1], AF.Sigmoid)
            # per-partition bias constants t_p for the ACT relu knots
            tkn = singles.tile([128, len(XKNOTS)], F32)
            for i, p in enumerate(XKNOTS):
                nc.vector.memset(tkn[:, i:i + 1], p * H)

            KTb = singles.tile([128, S], FP16)     # sigmoid K^T  [d, k]
            QTb = singles.tile([128, 256], FP16)   # sigmoid Q^T  [d, q]
            QCb = singles.tile([128, 256], FP16)   # 1 - Q  (= sigmoid(-z))
            Vn = singles.tile([128, 8, 128], FP16)  # V (no bv)  [k, blk, e]
            mp = singles.tile([128, P, S], FP16)    # moving feats min(K,t)-t
            vt = singles.tile([128, P, 256], FP16)  # stationary feats

            # ---- phase 1: warm-up + K/Q projections ----
            with (
                tc.tile_pool(name="pwu", bufs=1, space="PSUM") as pwu,
                tc.tile_pool(name="pproj", bufs=1, space="PSUM") as pproj,
            ):
                # dummy matmuls bridge the input-DMA wait so the PE HAM
                # clock-gate is released (2.4 GHz) before real work
                wups = pwu.tile([128, 512], F32, tag="wup")
                for i in range(16):
                    nc.tensor.matmul(
                        wups[:, 0:256], identity[:], warm512[:, 0:256],
                        start=True, stop=True)

                psq = pproj.tile([128, 256], F32, tag="projq")
                nc.tensor.matmul(psq[:], wq_sb, xqt_sb[:])
                nc.scalar.activation(
                    QTb[:], psq[:], AF.Sigmoid, bias=bq_sb, scale=1.0)
                psk = pproj.tile([128, S], F32, tag="proj")
                for hh in range(2):
                    nc.tensor.matmul(
                        psk[:, ts(hh, 512)], wk_sb, xbt_sb[:, ts(hh, 512)])
                    nc.scalar.activation(
                        KTb[:, ts(hh, 512)], psk[:, ts(hh, 512)],
                        AF.Sigmoid, bias=bk_sb, scale=1.0)

            # ---- phase 2+3+4: features, chunked score -> exp -> AV ----
            with (
                tc.tile_pool(name="utmp", bufs=4) as utp,
                tc.tile_pool(name="psc", bufs=1, space="PSUM") as psc,
                tc.tile_pool(name="ezt", bufs=1) as ezt,
                tc.tile_pool(name="sml", bufs=1) as sml,
                tc.tile_pool(name="wts", bufs=4) as wtsp,
                tc.tile_pool(name="ob", bufs=2) as ob,
            ):
                QN = utp.tile([128, 256], FP16, tag="qn")
                nc.vector.tensor_scalar(QN[:], QTb[:], -1.0, None, ALU.mult)
                for p in range(P):
                    t_p = p * H
                    if p in XKNOTS:
                        # moving on ACT: g_p = relu(t_p - K) >= 0
                        tp_ap = tkn[:, XKNOTS.index(p):XKNOTS.index(p) + 1]
                        for hh in range(2):
                            nc.scalar.activation(
                                mp[:, p, ts(hh, 512)], KTb[:, ts(hh, 512)],
                                AF.Relu, bias=tp_ap, scale=-1.0)
                        # stationary vtNEG_p = max(min(Q-t_p-h,0),
                        #                          min(-Q+t_p-h,0)) <= 0
                        a = utp.tile([128, 256], FP16, tag="ua")
                        b = utp.tile([128, 256], FP16, tag="ub")
                        nc.vector.tensor_scalar(
                            a[:], QTb[:], t_p + H, 0.0, ALU.subtract, ALU.min)
                        nc.vector.tensor_scalar(
                            b[:], QN[:], t_p - H, 0.0, ALU.add, ALU.min)
                        nc.vector.tensor_max(vt[:, p, :], a[:], b[:])
                    else:
                        # moving on DVE: m_p = min(K, t_p) - t_p <= 0
                        for hh in range(2):
                            nc.vector.tensor_scalar(
                                mp[:, p, ts(hh, 512)], KTb[:, ts(hh, 512)],
                                t_p, t_p, ALU.min, ALU.subtract)
                        # stationary vtPOS_p = min(relu(t_p+h-Q),
                        #                          relu(Q-t_p+h)) >= 0
                        a = utp.tile([128, 256], FP16, tag="ua")
                        b = utp.tile([128, 256], FP16, tag="ub")
                        nc.vector.tensor_scalar(
                            a[:], QN[:], t_p + H, 0.0, ALU.add, ALU.max)
                        nc.vector.tensor_scalar(
                            b[:], QTb[:], t_p - H, 0.0, ALU.subtract, ALU.max)
                        nc.vector.tensor_tensor(
                            vt[:, p, :], a[:], b[:], ALU.min)

                # V projection + copies; exp table preload
                with tc.tile_pool(name="pvv", bufs=1, space="PSUM") as pvv:
                    for half in range(2):
                        psv = pvv.tile([128, 4, 128], F32, tag="vv")
                        for j4 in range(4):
                            j = half * 4 + j4
                            nc.tensor.matmul(
                                psv[:, j4, :], xbt_sb[:, ts(j, 128)], wv_sb)
                        nc.scalar.copy(Vn[:, ts(half, 4), :], psv[:])
                # reads KTb so it data-depends on the LAST sigmoid --
                # otherwise the scheduler hoists it between the sigmoids
                # and the ACT table set thrashes (4 loads instead of 2)
                nc.scalar.activation(warm[:], KTb[:, 1023:1024], AF.Exp)

                pw_cm = tc.tile_pool(name="pw", bufs=3 if masked else 2,
                                     space="PSUM")
                po_cm = tc.tile_pool(name="po", bufs=1, space="PSUM")
                pw = pw_cm.__enter__()
                po = po_cm.__enter__()
                scA = psc.tile([128, WA], F32, tag="scA")
                scB = psc.tile([128, WB], F32, tag="scB")
                EA = ezt.tile([128, WA], FP16)
                EB = ezt.tile([128, WB], FP16)
                NCA, NCB = WA // 512, WB // 512
                rsA0 = sml.tile([128, 1], F32)
                rsA1 = sml.tile([128, 1], F32)
                rsB0 = sml.tile([128, 1], F32)
                rsB1 = sml.tile([128, 1], F32)
                rs2 = sml.tile([128, 1], F32)
                rs = {("A", 0): rsA0, ("A", 1): rsA1,
                      ("B", 0): rsB0, ("B", 1): rsB1}
                denA = sml.tile([128, 1], F32)
                denB = sml.tile([128, 1], F32)
                rcpA = sml.tile([128, 1], F32)
                rcpB = sml.tile([128, 1], F32)
                oA = po.tile([128, 128], F32, tag="oA")
                oB = po.tile([128, 128], F32, tag="oB")

                chunks = [("A", ca) for ca in range(NCA)]
                chunks += [("B", cb) for cb in range(NCB)]

                def emit_score(tile_id, ci):
                    sc = scA if tile_id == "A" else scB
                    qlo = 0 if tile_id == "A" else 128
                    for p in range(P):
                        nc.tensor.matmul(
                            sc[:, ts(ci, 512)],
                            vt[:, p, qlo:qlo + 128],
                            mp[:, p, ts(ci, 512)],
                            start=(p == 0), stop=(p == P - 1))

                def emit_zexp(tile_id, ci, nhalf):
                    sc, E = (scA, EA) if tile_id == "A" else (scB, EB)
                    m1 = m1a_sb if tile_id == "A" else m1b_sb
                    Z = utp.tile([128, 512], FP16, tag="z")
                    nc.vector.scalar_tensor_tensor(
                        out=Z[:], in0=sc[:, ts(ci, 512)], scalar=POS_DT,
                        in1=m1[:, ts(ci, 512)], op0=ALU.add, op1=ALU.mult)
                    rsc = rs[(tile_id, ci)]
                    for eh in range(nhalf):
                        w2 = 512 // nhalf
                        racc = rsc if eh == 0 else rs2
                        nc.scalar.activation(
                            E[:, ci * 512 + eh * w2:ci * 512 + (eh + 1) * w2],
                            Z[:, eh * w2:(eh + 1) * w2], AF.Exp,
                            scale=EXP_SCALE, accum_out=racc[:])
                    if nhalf == 2:
                        nc.vector.tensor_add(rsc[:], rsc[:], rs2[:])

                def emit_trav(tile_id, ci, close):
                    E = EA if tile_id == "A" else EB
                    o = oA if tile_id == "A" else oB
                    for j4 in range(4):
                        j = ci * 4 + j4
                        pwt = pw.tile([128, 128], FP16, tag="wt")
                        nc.tensor.transpose(
                            pwt[:], E[:, ts(j, 128)], identity[:])
                        wtile = wtsp.tile([128, 128], FP16, tag="wts")
                        if j % 2 == 0:
                            nc.scalar.copy(wtile[:], pwt[:])
                        else:
                            nc.vector.tensor_copy(wtile[:], pwt[:])
                        nc.tensor.matmul(
                            o[:], wtile[:], Vn[:, j, :],
                            start=(j == 0), stop=(close and j4 == 3))

                def emit_out(tile_id, o, rcp, r0, split=False):
                    ores = ob.tile([128, 128], F32, tag="ores")
                    if split:
                        # halves so the first store overlaps the second stt
                        nc.vector.scalar_tensor_tensor(
                            out=ores[:, 0:64], in0=o[:, 0:64], scalar=rcp[:],
                            in1=bvb_sb[:, 0:64], op0=ALU.mult, op1=ALU.add)
                        nc.sync.dma_start(
                            out=out_d[r0:r0 + 128, 0:64], in_=ores[:, 0:64])
                        nc.vector.scalar_tensor_tensor(
                            out=ores[:, 64:128], in0=o[:, 64:128],
                            scalar=rcp[:], in1=bvb_sb[:, 64:128],
                            op0=ALU.mult, op1=ALU.add)
                        nc.scalar.dma_start(
                            out=out_d[r0:r0 + 128, 64:128],
                            in_=ores[:, 64:128])
                    else:
                        nc.vector.scalar_tensor_tensor(
                            out=ores[:], in0=o[:], scalar=rcp[:],
                            in1=bvb_sb, op0=ALU.mult, op1=ALU.add)
                        nc.sync.dma_start(
                            out=out_d[r0:r0 + 128, :], in_=ores[:])

                def emit_a_epilogue():
                    if masked:
                        # masked tail keys 512..1023: weight-1
                        for j in range(4, 8):
                            nc.tensor.matmul(
                                oA[:], ones128[:], Vn[:, j, :],
                                start=False, stop=(j == 7))
                        nc.vector.tensor_scalar(
                            denA[:], rs[("A", 0)][:], float(S - WA),
                            None, ALU.add)
                    else:
                        nc.vector.tensor_add(
                            denA[:], rs[("A", 0)][:], rs[("A", 1)][:])
                    nc.vector.reciprocal(rcpA[:], denA[:])
                    emit_out("A", oA, rcpA, 0)

                # knot-major scores: every chunk advances in lockstep
                # with the feature ladder, so all chunks complete when the
                # last knot's features land (instead of B0/B1 re-sweeping
                # all knots serially after chunk A); B0/B1 share their
                # stationary per knot
                for p in range(P):
                    for (tile_id, ci) in chunks:
                        sc = scA if tile_id == "A" else scB
                        qlo = 0 if tile_id == "A" else 128
                        nc.tensor.matmul(
                            sc[:, ts(ci, 512)],
                            vt[:, p, qlo:qlo + 128],
                            mp[:, p, ts(ci, 512)],
                            start=(p == 0), stop=(p == P - 1))
                for (tile_id, ci) in chunks[:-1]:
                    last = ci == (NCA if tile_id == "A" else NCB) - 1
                    emit_zexp(tile_id, ci, 1)
                    emit_trav(tile_id, ci, close=(last and not
                                                  (tile_id == "A" and masked)))
                emit_zexp("B", NCB - 1, 2)
                emit_a_epilogue()
                emit_trav("B", NCB - 1, close=True)
                nc.vector.tensor_add(
                    denB[:], rs[("B", 0)][:], rs[("B", 1)][:])
                nc.vector.reciprocal(rcpB[:], denB[:])
                emit_out("B", oB, rcpB, 128, split=True)
                po_cm.__exit__(None, None, None)
                pw_cm.__exit__(None, None, None)

    nc.finalize()
    return nc


_PROG_CACHE: dict[bool, bass.Bass] = {}


def _get_program(masked: bool) -> bass.Bass:
    if masked not in _PROG_CACHE:
        _PROG_CACHE[masked] = _build_program(masked)
    return _PROG_CACHE[masked]


def build_in_maps(x, Wq, bq, Wk, bk, Wv, bv, masked):
    wkt = np.ascontiguousarray(Wk.T.astype(np.float16))
    wqt = np.ascontiguousarray(Wq.T.astype(np.float16))
    wvt = np.ascontiguousarray(Wv.T.astype(np.float16))
    bcat = np.ascontiguousarray(
        np.concatenate(
            [bk.reshape(D, 1), bq.reshape(D, 1), -bq.reshape(D, 1),
             np.tile(bv.reshape(1, D), (D, 1))], axis=1).astype(np.float32))
    WA = 512 if masked else 1024
    WB = 1024
    kidx = np.arange(S)
    in_maps = []
    for c in range(NCORES):
        b, l = divmod(c, 4)
        xb16 = x[b].astype(np.float16)
        xbt = np.ascontiguousarray(xb16.T)
        rows = np.concatenate(
            [128 * l + np.arange(128), 128 * (4 + l) + np.arange(128)])
        xqt = np.ascontiguousarray(xb16[rows].T)
        if masked:
            qa = (128 * l + np.arange(128))[:, None]
            qb = (128 * (4 + l) + np.arange(128))[:, None]
            m1a = (kidx[None, :WA] <= qa).astype(np.float16)
            m1b = (kidx[None, :WB] <= qb).astype(np.float16)
        else:
            m1a = np.ones((128, WA), np.float16)
            m1b = np.ones((128, WB), np.float16)
        in_maps.append({
            "xbt": xbt, "xqt": xqt, "wk": wkt, "wq": wqt, "wv": wvt,
            "bcat": bcat,
            "m1a": np.ascontiguousarray(m1a),
            "m1b": np.ascontiguousarray(m1b),
        })
    return in_maps


def assemble_out(results, masked):
    out = np.empty((B, S, D), dtype=np.float32)
    for c in range(NCORES):
        b, l = divmod(c, 4)
        res = results[c]["out"]
        out[b, 128 * l:128 * (l + 1)] = res[0:128]
        out[b, 128 * (4 + l):128 * (5 + l)] = res[128:256]
    return out


def kernel(x, Wq, bq, Wk, bk, Wv, bv, apply_causal_mask):
    x = np.ascontiguousarray(np.asarray(x, dtype=np.float32))
    Wq = np.asarray(Wq, dtype=np.float32)
    Wk = np.asarray(Wk, dtype=np.float32)
    Wv = np.asarray(Wv, dtype=np.float32)
    bq = np.asarray(bq, dtype=np.float32)
    bk = np.asarray(bk, dtype=np.float32)
    bv = np.asarray(bv, dtype=np.float32)
    masked = bool(int(np.asarray(apply_causal_mask)))

    nc = _get_program(masked)
    in_maps = build_in_maps(x, Wq, bq, Wk, bk, Wv, bv, masked)
    res = run_bass_kernel_spmd(nc, in_maps, list(range(NCORES))).results
    return assemble_out(res, masked)
